# revision 28
# baseline (speedup 1.0000x reference)
"""Multi-head attention (B=4, T=2048, D=1024, H=16, causal) on 8 TRN2 NeuronCores.

Sharding: tensor-parallel over heads — core c owns heads {2c, 2c+1}
(columns [128c, 128c+128) of the QKV projections, rows [128c, 128c+128) of Wo).
Each core computes q/k/v for its heads over all B*T tokens, causal attention,
and a partial output projection; the host sums the 8 partials and adds bo.

Layout: "feature-major" — activations kept as [feature, token] so every matmul
contracts over the partition dim without transposes.  Scores are computed
transposed (S_T[tk, tq]) so softmax needs no P transpose for P@V; the softmax
denominator comes free from a ones-column appended to V; normalization happens
after P@V on the small output tile.  Matmul inputs use float32r (TF32-like,
full bf16-rate on the PE at moving-dim >= 256, ~1.5e-4 matmul rel err).
"""
import sys

sys.path.insert(0, "/opt/trn_rl_repo")

import numpy as np

import concourse.bacc as bacc
import concourse.tile as tile
from concourse import mybir
from concourse.bass_utils import run_bass_kernel_spmd
from concourse.masks import make_identity

B, T, D, H, HD = 4, 2048, 1024, 16, 64
NCORES = 8
DPC = 128          # dout per core = 2 heads * 64
BT = B * T         # 8192
TW = 512           # tq window width
NTG = BT // TW     # 16 token groups
NKT = D // 128     # 8 contraction tiles for projections
NWIN = T // TW     # 4 tq windows per batch
VSTRIDE = 2 * (HD + 1)  # 130: per-tk-tile V_aug columns (2 heads x (64 V + 1 ones))
SCALE = 1.0 / np.sqrt(HD)

f32 = mybir.dt.float32
f32r = mybir.dt.float32r
MULT = mybir.AluOpType.mult

_cache = {}


def _build(with_bias: bool, debug: bool = False):
    nc = bacc.Bacc()
    xT = nc.dram_tensor("xT", [D, BT], f32r, kind="ExternalInput")
    wq = nc.dram_tensor("wq", [D, DPC], f32r, kind="ExternalInput")
    wk = nc.dram_tensor("wk", [D, DPC], f32r, kind="ExternalInput")
    wv = nc.dram_tensor("wv", [D, DPC], f32r, kind="ExternalInput")
    wo = nc.dram_tensor("wo", [DPC, D], f32r, kind="ExternalInput")
    out = nc.dram_tensor("out", [D, BT], mybir.dt.float16, kind="ExternalOutput")
    if debug:
        dbg_qT = nc.dram_tensor("dbg_qT", [128, BT], f32, kind="ExternalOutput")
        dbg_kT = nc.dram_tensor("dbg_kT", [128, BT], f32, kind="ExternalOutput")
        dbg_va = nc.dram_tensor("dbg_va", [128, (BT // 128) * VSTRIDE], f32, kind="ExternalOutput")
        dbg_oT = nc.dram_tensor("dbg_oT", [128, BT], f32, kind="ExternalOutput")
        dbg_ost = nc.dram_tensor("dbg_ost", [HD + 1, TW], f32, kind="ExternalOutput")
        dbg_rc = nc.dram_tensor("dbg_rc", [1, TW], f32, kind="ExternalOutput")
        dbg_bc = nc.dram_tensor("dbg_bc", [HD, TW], f32, kind="ExternalOutput")
    if with_bias:
        bq = nc.dram_tensor("bq", [DPC, 1], f32, kind="ExternalInput")
        bk = nc.dram_tensor("bk", [DPC, 1], f32, kind="ExternalInput")
        bv = nc.dram_tensor("bv", [DPC, 1], f32, kind="ExternalInput")

    # tri[p, f] = 1.0 if f >= p else 0.0 (keep iff tq >= tk on the diagonal block)
    tri_np = np.zeros((128, 128), dtype=np.float32)
    p_idx = np.arange(128)[:, None]
    f_idx = np.arange(128)[None, :]
    tri_np[f_idx >= p_idx] = 1.0
    tri_dram = nc.inline_tensor(tri_np, name="tri")

    with tile.TileContext(nc) as tc:
        with (
            tc.tile_pool(name="pers", bufs=1) as pers,
            tc.tile_pool(name="xp", bufs=3) as xp,
            tc.tile_pool(name="vs", bufs=2) as vsp,
            tc.tile_pool(name="pp", bufs=2) as ppool,
            tc.tile_pool(name="nrm", bufs=2) as nrm,
            tc.tile_pool(name="outp", bufs=3) as outp,
        ):
            wq_sb = pers.tile([128, D], f32r, tag="wq")
            wk_sb = pers.tile([128, D], f32r, tag="wk")
            wv_sb = pers.tile([128, D], f32r, tag="wv")
            wo_sb = pers.tile([128, D], f32r, tag="wo")
            qT = pers.tile([128, BT], f32r, tag="qT")
            kT = pers.tile([128, BT], f32r, tag="kT")
            oT = pers.tile([128, BT], f32r, tag="oT")
            vaug = pers.tile([128, (BT // 128) * VSTRIDE], f32r, tag="vaug")
            tri_sb = pers.tile([128, 128], f32r, tag="tri")
            ident = pers.tile([128, 128], f32, tag="ident")

            make_identity(nc, ident[:])
            nc.sync.dma_start(tri_sb[:], tri_dram[:].bitcast(f32r))
            # ones columns of V_aug (col 64 and 129 of each VSTRIDE block)
            vaug_ones = vaug[:].rearrange(
                "p (t g w) -> p t g w", t=BT // 128, g=2
            )[:, :, :, HD : HD + 1]
            nc.gpsimd.memset(vaug_ones.bitcast(f32), 1.0)
            for kt in range(NKT):
                s = slice(kt * 128, kt * 128 + 128)
                nc.sync.dma_start(wq_sb[:, s], wq[s, :])
                nc.sync.dma_start(wk_sb[:, s], wk[s, :])
                nc.sync.dma_start(wv_sb[:, s], wv[s, :])
            nc.sync.dma_start(wo_sb[:], wo[:, :])
            if with_bias:
                bq_sb = pers.tile([128, 1], f32, tag="bq")
                bk_sb = pers.tile([128, 1], f32, tag="bk")
                bv_sb = pers.tile([128, 1], f32, tag="bv")
                nc.sync.dma_start(bq_sb[:], bq[:, :])
                nc.sync.dma_start(bk_sb[:], bk[:, :])
                nc.sync.dma_start(bv_sb[:], bv[:, :])

            # ---- Phase 1: QKV projections (feature-major), V transposed ----
            # Token groups processed in pairs so each weight k-tile is reused
            # for two consecutive matmuls (halves the LDWEIGHTS serialization).
            with tc.tile_pool(name="ps1", bufs=1, space="PSUM") as ps1:
                for tgp in range(NTG // 2):
                    toks = [
                        slice((2 * tgp + i) * TW, (2 * tgp + i) * TW + TW)
                        for i in (0, 1)
                    ]
                    qkv_ps = {
                        (w, i): ps1.tile([128, TW], f32, tag=f"{w}{i}", name=f"{w}{i}")
                        for w in "qkv"
                        for i in (0, 1)
                    }
                    for kt in range(NKT):
                        s = slice(kt * 128, kt * 128 + 128)
                        x_t = xp.tile([128, 2 * TW], f32r, tag="x", name="x", bufs=3)
                        nc.sync.dma_start(
                            x_t[:], xT[s, 2 * tgp * TW : 2 * tgp * TW + 2 * TW]
                        )
                        st, sp = kt == 0, kt == NKT - 1
                        for w, w_sb in (("q", wq_sb), ("k", wk_sb), ("v", wv_sb)):
                            for i in (0, 1):
                                nc.tensor.matmul(
                                    qkv_ps[(w, i)][:], w_sb[:, s],
                                    x_t[:, i * TW : i * TW + TW],
                                    start=st, stop=sp,
                                )
                    for i in (0, 1):
                        tok = toks[i]
                        q_ps, k_ps, v_ps = (qkv_ps[(w, i)] for w in "qkv")
                        if with_bias:
                            nc.scalar.add(qT[:, tok], q_ps[:], bq_sb[:])
                            nc.scalar.add(kT[:, tok], k_ps[:], bk_sb[:])
                        else:
                            nc.scalar.copy(qT[:, tok], q_ps[:])
                            nc.scalar.copy(kT[:, tok], k_ps[:])
                        v_st = vsp.tile([128, TW], f32, tag="vst")
                        if with_bias:
                            nc.vector.tensor_scalar_add(v_st[:], v_ps[:], bv_sb[:])
                        else:
                            nc.vector.tensor_copy(v_st[:], v_ps[:])
                        vt_ps = ps1.tile([128, TW], f32, tag="vt", bufs=2)
                        for j4 in range(TW // 128):
                            nc.tensor.transpose(
                                vt_ps[:, j4 * 128 : j4 * 128 + 128],
                                v_st[:, j4 * 128 : j4 * 128 + 128],
                                ident[:],
                            )
                        tg = 2 * tgp + i
                        c = tg * 4 * VSTRIDE
                        dst = vaug[:, c : c + 4 * VSTRIDE].rearrange(
                            "p (t g w) -> p t g w", t=4, g=2
                        )[:, :, :, 0:HD]
                        srcv = vt_ps[:].rearrange("p (t g w) -> p t g w", t=4, g=2)
                        nc.vector.tensor_copy(dst, srcv)

            # ---- Phase 2: causal attention, transposed scores ----
            # Per tk-tile j: both heads' scores go into one 2-bank psum tile
            # (bufs=3 so the PE streams ahead of the exp), one exp covers both
            # heads, PV accumulates into per-head O psum.  O is copied out to
            # SBUF immediately so normalization stays off the PE critical path.
            with tc.tile_pool(name="ps2", bufs=1, space="PSUM") as ps2:
                for b in range(B):
                    tb = b * T
                    for wi in range(NWIN):
                        win = slice(tb + wi * TW, tb + wi * TW + TW)
                        jmax = 4 * wi + 4
                        o_ps = [
                            ps2.tile([HD + 1, TW], f32, tag=f"o{h}", name=f"o{h}")
                            for h in (0, 1)
                        ]
                        for j in range(jmax):
                            bj = slice(tb + j * 128, tb + j * 128 + 128)
                            s_pr = ps2.tile([128, 2 * TW], f32, tag="s", bufs=3)
                            for h in (0, 1):
                                nc.tensor.matmul(
                                    s_pr[:, h * TW : h * TW + TW],
                                    kT[h * HD : h * HD + HD, bj],
                                    qT[h * HD : h * HD + HD, win],
                                    start=True,
                                    stop=True,
                                )
                            p_pr = ppool.tile([128, 2 * TW], f32r, tag="p", bufs=3)
                            nc.scalar.activation(
                                p_pr[:],
                                s_pr[:],
                                mybir.ActivationFunctionType.Exp,
                                scale=float(SCALE),
                            )
                            d = j - 4 * wi
                            if d >= 0:  # diagonal tile: zero strict lower triangle
                                for h in (0, 1):
                                    ts = slice(
                                        h * TW + 128 * d, h * TW + 128 * d + 128
                                    )
                                    nc.vector.tensor_tensor(
                                        p_pr[:, ts], p_pr[:, ts], tri_sb[:], MULT
                                    )
                            c0 = 128 * d if d > 0 else 0
                            vcol = ((tb // 128) + j) * VSTRIDE
                            for h in (0, 1):
                                nc.tensor.matmul(
                                    o_ps[h][:, c0:TW],
                                    vaug[:, vcol + h * (HD + 1) : vcol + h * (HD + 1) + HD + 1],
                                    p_pr[:, h * TW + c0 : h * TW + TW],
                                    start=(j == 0),
                                    stop=(j == jmax - 1),
                                )
                        for h in (0, 1):
                            o_st = nrm.tile([HD + 1, TW], f32, tag="ost", bufs=3)
                            nc.vector.tensor_copy(o_st[:], o_ps[h][:])
                            den0 = nrm.tile([1, TW], f32, tag="den0")
                            nc.sync.dma_start(den0[:], o_st[HD : HD + 1, :])
                            bc = nrm.tile([HD, TW], f32, tag="bc")
                            nc.gpsimd.partition_broadcast(bc[:], den0[0:1, :])
                            rc = nrm.tile([HD, TW], f32, tag="rc")
                            nc.vector.reciprocal_approx_fast(out=rc[:], in_=bc[:])
                            nc.vector.tensor_tensor(
                                oT[h * HD : h * HD + HD, win], o_st[0:HD, :], rc[:], MULT
                            )
                            if debug and b == 1 and wi == 2 and h == 0:
                                nc.sync.dma_start(dbg_ost[:], o_st[:])
                                nc.sync.dma_start(dbg_rc[:], rc[0:1, :])
                                nc.sync.dma_start(dbg_bc[:], bc[:])

                        # out-projection for this window's tokens (oT slice is
                        # final once both heads are normalized)
                        tg = 4 * b + wi
                        tok = slice(tg * TW, tg * TW + TW)
                        for dmg in range(4):
                            pr = ps2.tile(
                                [128, 2 * TW], f32, tag="s", name="pr", bufs=3
                            )
                            for dmi in range(2):
                                dm = dmg * 2 + dmi
                                s = slice(dm * 128, dm * 128 + 128)
                                nc.tensor.matmul(
                                    pr[:, dmi * TW : dmi * TW + TW],
                                    wo_sb[:, s],
                                    oT[:, tok],
                                    start=True,
                                    stop=True,
                                )
                            st2 = outp.tile([128, 2 * TW], mybir.dt.float16, tag="st")
                            nc.vector.tensor_copy(st2[:, 0:TW], pr[:, 0:TW])
                            nc.scalar.copy(st2[:, TW : 2 * TW], pr[:, TW : 2 * TW])
                            for dmi in range(2):
                                dm = dmg * 2 + dmi
                                s = slice(dm * 128, dm * 128 + 128)
                                nc.sync.dma_start(
                                    out[s, tok], st2[:, dmi * TW : dmi * TW + TW]
                                )

            if debug:
                with tc.tile_pool(name="dbgp", bufs=2) as dbgp:
                    for tg in range(NTG):
                        tok = slice(tg * TW, tg * TW + TW)
                        for name, sbuf, dram in (
                            ("q", qT, dbg_qT), ("k", kT, dbg_kT), ("o", oT, dbg_oT)
                        ):
                            t = dbgp.tile([128, TW], f32, tag="d", name="d")
                            nc.vector.tensor_copy(t[:], sbuf[:, tok].bitcast(f32))
                            nc.sync.dma_start(dram[:, tok], t[:])
                    for c0 in range(0, (BT // 128) * VSTRIDE, 520):
                        w = min(520, (BT // 128) * VSTRIDE - c0)
                        t = dbgp.tile([128, 520], f32, tag="d", name="d")
                        nc.vector.tensor_copy(t[:, 0:w], vaug[:, c0 : c0 + w].bitcast(f32))
                        nc.sync.dma_start(dbg_va[:, c0 : c0 + w], t[:, 0:w])

    nc.compile()
    return nc


def _get_nc(with_bias: bool):
    key = with_bias
    if key not in _cache:
        _cache[key] = _build(with_bias)
    return _cache[key]


def _make_in_maps(x, Wq, bq, Wk, bk, Wv, bv, Wo, with_bias):
    xT = np.ascontiguousarray(x.reshape(BT, D).T)
    in_maps = []
    for c in range(NCORES):
        cs = slice(c * DPC, c * DPC + DPC)
        m = {
            "xT": xT,
            "wq": np.ascontiguousarray(Wq[:, cs]),
            "wk": np.ascontiguousarray(Wk[:, cs]),
            "wv": np.ascontiguousarray(Wv[:, cs]),
            "wo": np.ascontiguousarray(Wo[cs, :]),
        }
        if with_bias:
            m["bq"] = np.ascontiguousarray(bq[cs]).reshape(DPC, 1)
            m["bk"] = np.ascontiguousarray(bk[cs]).reshape(DPC, 1)
            m["bv"] = np.ascontiguousarray(bv[cs]).reshape(DPC, 1)
        in_maps.append(m)
    return in_maps


def _gather(res, bo):
    acc = np.zeros((D, BT), dtype=np.float32)
    for r in res.results:
        acc += r["out"].astype(np.float32)
    y = acc.T + bo[None, :]
    return np.ascontiguousarray(y.reshape(B, T, D), dtype=np.float32)


def kernel(x, Wq, bq, Wk, bk, Wv, bv, Wo, bo, _trace=False):
    x = np.asarray(x, dtype=np.float32)
    Wq, Wk, Wv, Wo = (np.asarray(w, dtype=np.float32) for w in (Wq, Wk, Wv, Wo))
    bq, bk, bv, bo = (np.asarray(b_, dtype=np.float32) for b_ in (bq, bk, bv, bo))

    with_bias = bool(np.any(bq != 0) or np.any(bk != 0) or np.any(bv != 0))
    nc = _get_nc(with_bias)
    in_maps = _make_in_maps(x, Wq, bq, Wk, bk, Wv, bv, Wo, with_bias)
    res = run_bass_kernel_spmd(
        nc, in_maps, core_ids=list(range(NCORES)), trace=_trace
    )
    y = _gather(res, bo)
    if _trace:
        return y, res
    return y


# revision 29
# speedup vs baseline: 1.2404x; 1.2404x over previous
"""Multi-head attention (B=4, T=2048, D=1024, H=16, causal) on 8 TRN2 NeuronCores.

Sharding: tensor-parallel over heads — core c owns heads {2c, 2c+1}
(columns [128c, 128c+128) of the QKV projections, rows [128c, 128c+128) of Wo).
Each core computes q/k/v for its heads over all B*T tokens, causal attention,
and a partial output projection; the host sums the 8 partials and adds bo.

Layout: "feature-major" — activations kept as [feature, token] so every matmul
contracts over the partition dim without transposes.  Scores are computed
transposed (S_T[tk, tq]) so softmax needs no P transpose for P@V; the softmax
denominator comes free from a ones-column appended to V; normalization happens
after P@V on the small output tile.  Matmul inputs use float32r (TF32-like,
full bf16-rate on the PE at moving-dim >= 256, ~1.5e-4 matmul rel err).
"""
import sys

sys.path.insert(0, "/opt/trn_rl_repo")

import numpy as np

import concourse.bacc as bacc
import concourse.tile as tile
from concourse import mybir
from concourse.bass_utils import run_bass_kernel_spmd
from concourse.masks import make_identity

B, T, D, H, HD = 4, 2048, 1024, 16, 64
NCORES = 8
DPC = 128          # dout per core = 2 heads * 64
BT = B * T         # 8192
TW = 512           # tq window width
NTG = BT // TW     # 16 token groups
NKT = D // 128     # 8 contraction tiles for projections
NWIN = T // TW     # 4 tq windows per batch
VSTRIDE = 2 * (HD + 1)  # 130: per-tk-tile V_aug columns (2 heads x (64 V + 1 ones))
SCALE = 1.0 / np.sqrt(HD)

f32 = mybir.dt.float32
f32r = mybir.dt.float32r
MULT = mybir.AluOpType.mult

_cache = {}


def _build(with_bias: bool, debug: bool = False):
    nc = bacc.Bacc()
    xT = nc.dram_tensor("xT", [D, BT], f32r, kind="ExternalInput")
    wq = nc.dram_tensor("wq", [D, DPC], f32r, kind="ExternalInput")
    wk = nc.dram_tensor("wk", [D, DPC], f32r, kind="ExternalInput")
    wv = nc.dram_tensor("wv", [D, DPC], f32r, kind="ExternalInput")
    wo = nc.dram_tensor("wo", [DPC, D], f32r, kind="ExternalInput")
    out = nc.dram_tensor("out", [D, BT], mybir.dt.float16, kind="ExternalOutput")
    if debug:
        dbg_qT = nc.dram_tensor("dbg_qT", [128, BT], f32, kind="ExternalOutput")
        dbg_kT = nc.dram_tensor("dbg_kT", [128, BT], f32, kind="ExternalOutput")
        dbg_va = nc.dram_tensor("dbg_va", [128, (BT // 128) * VSTRIDE], f32, kind="ExternalOutput")
        dbg_oT = nc.dram_tensor("dbg_oT", [128, BT], f32, kind="ExternalOutput")
        dbg_ost = nc.dram_tensor("dbg_ost", [HD + 1, TW], f32, kind="ExternalOutput")
        dbg_rc = nc.dram_tensor("dbg_rc", [1, TW], f32, kind="ExternalOutput")
        dbg_bc = nc.dram_tensor("dbg_bc", [HD, TW], f32, kind="ExternalOutput")
    if with_bias:
        bq = nc.dram_tensor("bq", [DPC, 1], f32, kind="ExternalInput")
        bk = nc.dram_tensor("bk", [DPC, 1], f32, kind="ExternalInput")
        bv = nc.dram_tensor("bv", [DPC, 1], f32, kind="ExternalInput")

    # tri[p, f] = 1.0 if f >= p else 0.0 (keep iff tq >= tk on the diagonal block)
    tri_np = np.zeros((128, 128), dtype=np.float32)
    p_idx = np.arange(128)[:, None]
    f_idx = np.arange(128)[None, :]
    tri_np[f_idx >= p_idx] = 1.0
    tri_dram = nc.inline_tensor(tri_np, name="tri")

    with tile.TileContext(nc) as tc:
        with (
            tc.tile_pool(name="pers", bufs=1) as pers,
            tc.tile_pool(name="xp", bufs=3) as xp,
            tc.tile_pool(name="vs", bufs=2) as vsp,
            tc.tile_pool(name="pp", bufs=2) as ppool,
            tc.tile_pool(name="nrm", bufs=2) as nrm,
            tc.tile_pool(name="outp", bufs=3) as outp,
        ):
            wq_sb = pers.tile([128, D], f32r, tag="wq")
            wk_sb = pers.tile([128, D], f32r, tag="wk")
            wv_sb = pers.tile([128, D], f32r, tag="wv")
            wo_sb = pers.tile([128, D], f32r, tag="wo")
            qT = pers.tile([128, BT], f32r, tag="qT")
            kT = pers.tile([128, BT], f32r, tag="kT")
            oT = pers.tile([128, BT], f32r, tag="oT")
            vaug = pers.tile([128, (BT // 128) * VSTRIDE], f32r, tag="vaug")
            tri_sb = pers.tile([128, 128], f32r, tag="tri")
            ident = pers.tile([128, 128], f32, tag="ident")

            make_identity(nc, ident[:])
            nc.sync.dma_start(tri_sb[:], tri_dram[:].bitcast(f32r))
            # ones columns of V_aug (col 64 and 129 of each VSTRIDE block)
            vaug_ones = vaug[:].rearrange(
                "p (t g w) -> p t g w", t=BT // 128, g=2
            )[:, :, :, HD : HD + 1]
            nc.gpsimd.memset(vaug_ones.bitcast(f32), 1.0)
            for kt in range(NKT):
                s = slice(kt * 128, kt * 128 + 128)
                nc.sync.dma_start(wq_sb[:, s], wq[s, :])
                nc.sync.dma_start(wk_sb[:, s], wk[s, :])
                nc.sync.dma_start(wv_sb[:, s], wv[s, :])
            nc.sync.dma_start(wo_sb[:], wo[:, :])
            if with_bias:
                bq_sb = pers.tile([128, 1], f32, tag="bq")
                bk_sb = pers.tile([128, 1], f32, tag="bk")
                bv_sb = pers.tile([128, 1], f32, tag="bv")
                nc.sync.dma_start(bq_sb[:], bq[:, :])
                nc.sync.dma_start(bk_sb[:], bk[:, :])
                nc.sync.dma_start(bv_sb[:], bv[:, :])

            # ---- Phase 1: QKV projections (feature-major), V transposed ----
            # Token groups processed in pairs so each weight k-tile is reused
            # for two consecutive matmuls (halves the LDWEIGHTS serialization).
            with tc.tile_pool(name="ps1", bufs=1, space="PSUM") as ps1:
                for tgp in range(NTG // 2):
                    toks = [
                        slice((2 * tgp + i) * TW, (2 * tgp + i) * TW + TW)
                        for i in (0, 1)
                    ]
                    qkv_ps = {
                        (w, i): ps1.tile([128, TW], f32, tag=f"{w}{i}", name=f"{w}{i}")
                        for w in "qkv"
                        for i in (0, 1)
                    }
                    for kt in range(NKT):
                        s = slice(kt * 128, kt * 128 + 128)
                        x_t = xp.tile([128, 2 * TW], f32r, tag="x", name="x", bufs=3)
                        nc.sync.dma_start(
                            x_t[:], xT[s, 2 * tgp * TW : 2 * tgp * TW + 2 * TW]
                        )
                        st, sp = kt == 0, kt == NKT - 1
                        for w, w_sb in (("q", wq_sb), ("k", wk_sb), ("v", wv_sb)):
                            for i in (0, 1):
                                nc.tensor.matmul(
                                    qkv_ps[(w, i)][:], w_sb[:, s],
                                    x_t[:, i * TW : i * TW + TW],
                                    start=st, stop=sp,
                                )
                    for i in (0, 1):
                        tok = toks[i]
                        q_ps, k_ps, v_ps = (qkv_ps[(w, i)] for w in "qkv")
                        if with_bias:
                            nc.scalar.add(qT[:, tok], q_ps[:], bq_sb[:])
                            nc.scalar.add(kT[:, tok], k_ps[:], bk_sb[:])
                        else:
                            nc.scalar.copy(qT[:, tok], q_ps[:])
                            nc.scalar.copy(kT[:, tok], k_ps[:])
                        v_st = vsp.tile([128, TW], f32, tag="vst")
                        if with_bias:
                            nc.vector.tensor_scalar_add(v_st[:], v_ps[:], bv_sb[:])
                        else:
                            nc.vector.tensor_copy(v_st[:], v_ps[:])
                        vt_ps = ps1.tile([128, TW], f32, tag="vt", bufs=2)
                        for j4 in range(TW // 128):
                            nc.tensor.transpose(
                                vt_ps[:, j4 * 128 : j4 * 128 + 128],
                                v_st[:, j4 * 128 : j4 * 128 + 128],
                                ident[:],
                            )
                        tg = 2 * tgp + i
                        c = tg * 4 * VSTRIDE
                        dst = vaug[:, c : c + 4 * VSTRIDE].rearrange(
                            "p (t g w) -> p t g w", t=4, g=2
                        )[:, :, :, 0:HD]
                        srcv = vt_ps[:].rearrange("p (t g w) -> p t g w", t=4, g=2)
                        nc.vector.tensor_copy(dst, srcv)

            # ---- Phase 2: causal attention, transposed scores ----
            # Per tk-tile j: both heads' scores go into one 2-bank psum tile
            # (bufs=3 so the PE streams ahead of the exp), one exp covers both
            # heads, PV accumulates into per-head O psum.  O is copied out to
            # SBUF immediately so normalization stays off the PE critical path.
            with tc.tile_pool(name="ps2", bufs=1, space="PSUM") as ps2:
                for b in range(B):
                    tb = b * T
                    for wi in range(NWIN):
                        win = slice(tb + wi * TW, tb + wi * TW + TW)
                        jmax = 4 * wi + 4
                        o_ps = [
                            ps2.tile([HD + 1, TW], f32, tag=f"o{h}", name=f"o{h}")
                            for h in (0, 1)
                        ]
                        for j in range(jmax):
                            bj = slice(tb + j * 128, tb + j * 128 + 128)
                            s_pr = ps2.tile([128, 2 * TW], f32, tag="s", bufs=3)
                            for h in (0, 1):
                                nc.tensor.matmul(
                                    s_pr[:, h * TW : h * TW + TW],
                                    kT[h * HD : h * HD + HD, bj],
                                    qT[h * HD : h * HD + HD, win],
                                    start=True,
                                    stop=True,
                                )
                            p_pr = ppool.tile([128, 2 * TW], f32r, tag="p", bufs=3)
                            nc.scalar.activation(
                                p_pr[:],
                                s_pr[:],
                                mybir.ActivationFunctionType.Exp,
                                scale=float(SCALE),
                            )
                            d = j - 4 * wi
                            if d >= 0:  # diagonal tile: zero strict lower triangle
                                for h in (0, 1):
                                    ts = slice(
                                        h * TW + 128 * d, h * TW + 128 * d + 128
                                    )
                                    nc.vector.tensor_tensor(
                                        p_pr[:, ts], p_pr[:, ts], tri_sb[:], MULT
                                    )
                            c0 = 128 * d if d > 0 else 0
                            vcol = ((tb // 128) + j) * VSTRIDE
                            for h in (0, 1):
                                nc.tensor.matmul(
                                    o_ps[h][:, c0:TW],
                                    vaug[:, vcol + h * (HD + 1) : vcol + h * (HD + 1) + HD + 1],
                                    p_pr[:, h * TW + c0 : h * TW + TW],
                                    start=(j == 0),
                                    stop=(j == jmax - 1),
                                )
                        for h in (0, 1):
                            o_st = nrm.tile([HD + 1, TW], f32, tag="ost", bufs=3)
                            nc.vector.tensor_copy(o_st[:], o_ps[h][:])
                            den0 = nrm.tile([1, TW], f32, tag="den0")
                            nc.sync.dma_start(den0[:], o_st[HD : HD + 1, :])
                            bc = nrm.tile([HD, TW], f32, tag="bc")
                            nc.gpsimd.partition_broadcast(bc[:], den0[0:1, :])
                            rc = nrm.tile([HD, TW], f32, tag="rc")
                            nc.vector.reciprocal_approx_fast(out=rc[:], in_=bc[:])
                            nc.vector.tensor_tensor(
                                oT[h * HD : h * HD + HD, win], o_st[0:HD, :], rc[:], MULT
                            )
                            if debug and b == 1 and wi == 2 and h == 0:
                                nc.sync.dma_start(dbg_ost[:], o_st[:])
                                nc.sync.dma_start(dbg_rc[:], rc[0:1, :])
                                nc.sync.dma_start(dbg_bc[:], bc[:])

                    # ---- partial output projection for this batch ----
                    for tg in range(4 * b, 4 * b + 4):
                        tok = slice(tg * TW, tg * TW + TW)
                        for dmg in range(4):
                            pr = ps2.tile([128, 2 * TW], f32, tag="s", name="pr", bufs=3)
                            for dmi in range(2):
                                dm = dmg * 2 + dmi
                                s = slice(dm * 128, dm * 128 + 128)
                                nc.tensor.matmul(
                                    pr[:, dmi * TW : dmi * TW + TW],
                                    wo_sb[:, s],
                                    oT[:, tok],
                                    start=True,
                                    stop=True,
                                )
                            st2 = outp.tile([128, 2 * TW], mybir.dt.float16, tag="st")
                            nc.vector.tensor_copy(st2[:, 0:TW], pr[:, 0:TW])
                            nc.scalar.copy(st2[:, TW : 2 * TW], pr[:, TW : 2 * TW])
                            for dmi in range(2):
                                dm = dmg * 2 + dmi
                                s = slice(dm * 128, dm * 128 + 128)
                                nc.sync.dma_start(
                                    out[s, tok], st2[:, dmi * TW : dmi * TW + TW]
                                )

            if debug:
                with tc.tile_pool(name="dbgp", bufs=2) as dbgp:
                    for tg in range(NTG):
                        tok = slice(tg * TW, tg * TW + TW)
                        for name, sbuf, dram in (
                            ("q", qT, dbg_qT), ("k", kT, dbg_kT), ("o", oT, dbg_oT)
                        ):
                            t = dbgp.tile([128, TW], f32, tag="d", name="d")
                            nc.vector.tensor_copy(t[:], sbuf[:, tok].bitcast(f32))
                            nc.sync.dma_start(dram[:, tok], t[:])
                    for c0 in range(0, (BT // 128) * VSTRIDE, 520):
                        w = min(520, (BT // 128) * VSTRIDE - c0)
                        t = dbgp.tile([128, 520], f32, tag="d", name="d")
                        nc.vector.tensor_copy(t[:, 0:w], vaug[:, c0 : c0 + w].bitcast(f32))
                        nc.sync.dma_start(dbg_va[:, c0 : c0 + w], t[:, 0:w])

    nc.compile()
    return nc


def _get_nc(with_bias: bool):
    key = with_bias
    if key not in _cache:
        _cache[key] = _build(with_bias)
    return _cache[key]


def _make_in_maps(x, Wq, bq, Wk, bk, Wv, bv, Wo, with_bias):
    xT = np.ascontiguousarray(x.reshape(BT, D).T)
    in_maps = []
    for c in range(NCORES):
        cs = slice(c * DPC, c * DPC + DPC)
        m = {
            "xT": xT,
            "wq": np.ascontiguousarray(Wq[:, cs]),
            "wk": np.ascontiguousarray(Wk[:, cs]),
            "wv": np.ascontiguousarray(Wv[:, cs]),
            "wo": np.ascontiguousarray(Wo[cs, :]),
        }
        if with_bias:
            m["bq"] = np.ascontiguousarray(bq[cs]).reshape(DPC, 1)
            m["bk"] = np.ascontiguousarray(bk[cs]).reshape(DPC, 1)
            m["bv"] = np.ascontiguousarray(bv[cs]).reshape(DPC, 1)
        in_maps.append(m)
    return in_maps


def _gather(res, bo):
    acc = np.zeros((D, BT), dtype=np.float32)
    for r in res.results:
        acc += r["out"].astype(np.float32)
    y = acc.T + bo[None, :]
    return np.ascontiguousarray(y.reshape(B, T, D), dtype=np.float32)


def kernel(x, Wq, bq, Wk, bk, Wv, bv, Wo, bo, _trace=False):
    x = np.asarray(x, dtype=np.float32)
    Wq, Wk, Wv, Wo = (np.asarray(w, dtype=np.float32) for w in (Wq, Wk, Wv, Wo))
    bq, bk, bv, bo = (np.asarray(b_, dtype=np.float32) for b_ in (bq, bk, bv, bo))

    with_bias = bool(np.any(bq != 0) or np.any(bk != 0) or np.any(bv != 0))
    nc = _get_nc(with_bias)
    in_maps = _make_in_maps(x, Wq, bq, Wk, bk, Wv, bv, Wo, with_bias)
    res = run_bass_kernel_spmd(
        nc, in_maps, core_ids=list(range(NCORES)), trace=_trace
    )
    y = _gather(res, bo)
    if _trace:
        return y, res
    return y
